# revision 22
# baseline (speedup 1.0000x reference)
"""Trainium2 Bass kernel for the LUT-linear (embedding_lookup) problem.

Math: per_table[b,t] = sum_c lut[t,c] * prod_j (1 + s_{c,j} x_j)/2 with
x_0 = input[b, mask[2t]], x_1 = input[b, mask[2t+1]], K=2 (KK=4 corners).
Expanding the corner products (codes s in {-1,+1}):
    per_table = a_t + b_t x0 + c_t x1 + d_t x0 x1
    4a = w0+w1+w2+w3, 4b = -w0+w1-w2+w3, 4c = -w0-w1+w2+w3, 4d = w0-w1-w2+w3
out[b,o] = bias[o] + sum_{t in seg_o} per_table   (segments are 512 contiguous
tables per out-feature).

Device strategy (8 NeuronCores, table-sharded; input replicated). The kernel
is SWDGE descriptor-rate bound (~10-12 ns/descriptor per queue, 4 queues), so
the layout is chosen to minimize descriptor count:
  - Tables are grouped (host-side, index-only planning) into single-feature
    slots of width 16: slot (p, k) holds up to 16 tables sharing one x0
    feature. x0 then needs only one 256B gather descriptor PER SLOT (2304)
    instead of per table (32768); inside the compute it is a stride-0
    broadcast view along the slot width. Only x1 is gathered per table
    (36864 descs incl. slot padding). Total 39168 descs vs 65536 for the
    naive both-gathers layout (201.5us naive -> 160-178us this version,
    rel err 3.3e-3 vs the 2e-2 gate; run-to-run HW variance is +/-15us).
  - Slots mix out-features, so the segment reduction cannot use a single
    partition-pairing matmul. Instead PE does the reduction per w-column
    with 0/1 host-built matrices: psum[o,b] += pmw[:,w,:]^T @ y[:,w,:].
    ldweights+matmul pairs sustain ~53ns on TRN2, 288 pairs ~= 15us.
  - DVE computes y = (d*x1 + b)*x0 + (c*x1 + a) in bf16. The b/c/d
    coefficient broadcasts are materialized as [128, W, B] tiles on the
    otherwise-idle Act engine so the DVE tensor_tensor ops all hit the
    2x 16-bit mode (a stride-0 last-dim broadcast operand forces 1x).
    Act also does the f32->bf16 casts (contiguous only - transposed or
    strided-column writes on Act/DVE are 3-5x slower). 18 half-size
    chunks (W=16) shrink the pipeline fill/drain tails.
  - Host does only index planning / layout transforms (permute, pad, cast,
    one-hot from indices); all value arithmetic stays on device.
"""

import numpy as np

NCORES = 8
B = 64
IN = 512
OUT = 512
T = IN * OUT
TC = T // NCORES          # tables per core = 32768
SEG = 512                 # tables per out-feature
OC = OUT // NCORES        # out-features per core = 64
NPART = 128

# slot layout
WSLOT = 16                # tables per single-feature slot
NPASS = 18                # slot columns per partition (worst core needs 2299 slots)
NSLOT = NPASS * NPART     # 2304 slots
WT2 = NPASS * WSLOT       # 288 w-positions per partition
NCHUNK = 18
W = WT2 // NCHUNK         # 16 w per chunk (= 1 pass)
KPC = NPASS // NCHUNK           # slot columns per chunk = 1
GIDX = 1024               # indices per dma_gather (ucode limit)
GSUB = NPART * W // GIDX  # x1 sub-gathers per chunk = 4
NQUEUES = 4

_CACHE = {}


def _build_program():
    import concourse.bacc as bacc
    import concourse.mybir as mybir
    from concourse import library_config
    from concourse.tile import TileContext

    f32 = mybir.dt.float32
    bf16 = mybir.dt.bfloat16
    i16 = mybir.dt.int16
    Alu = mybir.AluOpType
    SP = False

    nc = bacc.Bacc("TRN2", target_bir_lowering=False, debug=False,
                   num_devices=NCORES, num_swdge_queues=NQUEUES,
                   dynamic_dma_scratch_size=32768)

    input_t = nc.dram_tensor("input_t", [IN, B], f32, kind="ExternalInput")
    # x1 idx: per (chunk, sub) 64 wrapped cols
    idx1_d = nc.dram_tensor("idx1", [NPART, NCHUNK * GSUB * (GIDX // 16)],
                            i16, kind="ExternalInput")
    # x0 slot idx: 1024+1024+256 idxs -> 64+64+16 wrapped cols
    idx0s_d = nc.dram_tensor("idx0s", [NPART, 144], i16, kind="ExternalInput")
    lutp_d = nc.dram_tensor("lutp", [NCHUNK, NPART, W * 4], f32,
                            kind="ExternalInput")
    pmw_d = nc.dram_tensor("pmw", [NCHUNK, NPART, W * OC], bf16,
                           kind="ExternalInput")
    bias_d = nc.dram_tensor("bias_sh", [OC, 1], f32, kind="ExternalInput")
    out_d = nc.dram_tensor("out_c", [OC, B], f32, kind="ExternalOutput")

    with TileContext(nc) as tc:
        nc.gpsimd.load_library(library_config.mlp)
        with (
            tc.tile_pool(name="idx", bufs=1) as idx_pool,
            tc.tile_pool(name="small", bufs=1) as small_pool,
            tc.tile_pool(name="lut", bufs=2) as lut_pool,
            tc.tile_pool(name="pmw", bufs=2) as pmw_pool,
            tc.tile_pool(name="coef", bufs=2) as coef_pool,
            tc.tile_pool(name="xs", bufs=1) as xs_pool,
            tc.tile_pool(name="x1", bufs=6) as x1_pool,
            tc.tile_pool(name="m", bufs=3) as m_pool,
            tc.tile_pool(name="cm", bufs=2) as cm_pool,
            tc.tile_pool(name="psum", bufs=1, space="PSUM") as psum_pool,
        ):
            idx1_sb = idx_pool.tile([NPART, NCHUNK * GSUB * 64], i16, tag="idx1")
            idx0s_sb = idx_pool.tile([NPART, 144], i16, tag="idx0s")
            nc.sync.dma_start(idx1_sb[:], idx1_d[:])
            nc.sync.dma_start(idx0s_sb[:], idx0s_d[:])

            # x0 slot values: one descriptor per slot
            xsl = xs_pool.tile([NPART, NPASS, B], f32, tag="xsl")
            nc.gpsimd.dma_gather(xsl[:, 0:8, :], input_t[:],
                                 idx0s_sb[:, 0:64], GIDX, GIDX, B,
                                 queue_num=0, single_packet=SP)
            nc.gpsimd.dma_gather(xsl[:, 8:16, :], input_t[:],
                                 idx0s_sb[:, 64:128], GIDX, GIDX, B,
                                 queue_num=1, single_packet=SP)
            nc.gpsimd.dma_gather(xsl[:, 16:18, :], input_t[:],
                                 idx0s_sb[:, 128:144], 256, 256, B,
                                 queue_num=2, single_packet=SP)
            xslb = xs_pool.tile([NPART, NPASS, B], bf16, tag="xslb")
            nc.scalar.copy(xslb[:], xsl[:])

            bias_sb = small_pool.tile([OC, 1], f32, tag="bias")
            nc.sync.dma_start(bias_sb[:], bias_d[:])

            ps = psum_pool.tile([OC, B], f32, tag="ps")

            for c in range(NCHUNK):
                w4 = lut_pool.tile([NPART, W, 4], f32, tag="w4")
                nc.sync.dma_start(w4[:], lutp_d[c].rearrange("p (w k) -> p w k", k=4))
                pmw_sb = pmw_pool.tile([NPART, W, OC], bf16, tag="pmw")
                nc.sync.dma_start(pmw_sb[:], pmw_d[c].rearrange("p (w o) -> p w o", o=OC))

                # coefficient transform -> bf16 (values are 4x true a,b,c,d;
                # folded back by the 0.25 output scale)
                ca = coef_pool.tile([NPART, W], bf16, tag="ca")
                cb = coef_pool.tile([NPART, W], bf16, tag="cb")
                cc = coef_pool.tile([NPART, W], bf16, tag="cc")
                cd = coef_pool.tile([NPART, W], bf16, tag="cd")
                t1 = coef_pool.tile([NPART, W], f32, tag="t1")
                t2 = coef_pool.tile([NPART, W], f32, tag="t2")
                nc.vector.tensor_tensor(t1[:], w4[:, :, 0], w4[:, :, 3], Alu.add)
                nc.vector.tensor_tensor(t2[:], w4[:, :, 1], w4[:, :, 2], Alu.add)
                nc.vector.tensor_tensor(ca[:], t1[:], t2[:], Alu.add)
                nc.vector.tensor_tensor(cd[:], t1[:], t2[:], Alu.subtract)
                nc.vector.tensor_tensor(t1[:], w4[:, :, 3], w4[:, :, 0], Alu.subtract)
                nc.vector.tensor_tensor(t2[:], w4[:, :, 1], w4[:, :, 2], Alu.subtract)
                nc.vector.tensor_tensor(cb[:], t1[:], t2[:], Alu.add)
                nc.vector.tensor_tensor(cc[:], t1[:], t2[:], Alu.subtract)

                # x1 gathers (4 x 1024 descs)
                x1 = x1_pool.tile([NPART, W, B], f32, tag="x1")
                for j in range(GSUB):
                    q = (c * GSUB + j) % NQUEUES
                    nc.gpsimd.dma_gather(
                        x1[:, j * 8:(j + 1) * 8, :], input_t[:],
                        idx1_sb[:, (c * GSUB + j) * 64:(c * GSUB + j + 1) * 64], GIDX, GIDX, B,
                        queue_num=q, single_packet=SP)

                x1b = m_pool.tile([NPART, W, B], bf16, tag="x1b")
                nc.scalar.copy(x1b[:], x1[:])

                # Act materializes the two hot coefficient broadcasts so
                # their DVE ops run in the 2x 16-bit mode.
                bcdm = cm_pool.tile([NPART, W, B], bf16, tag="bcdm")
                bccm = cm_pool.tile([NPART, W, B], bf16, tag="bccm")
                bcbm = cm_pool.tile([NPART, W, B], bf16, tag="bcbm")
                nc.scalar.copy(bcdm[:], cd[:].unsqueeze(2).broadcast_to([NPART, W, B]))
                nc.scalar.copy(bccm[:], cc[:].unsqueeze(2).broadcast_to([NPART, W, B]))
                nc.scalar.copy(bcbm[:], cb[:].unsqueeze(2).broadcast_to([NPART, W, B]))

                # y = (d*x1 + b) * x0 + c*x1
                u = m_pool.tile([NPART, W, B], bf16, tag="u")
                v = m_pool.tile([NPART, W, B], bf16, tag="v")
                bca = ca[:].unsqueeze(2).broadcast_to([NPART, W, B])
                nc.vector.tensor_tensor(u[:], x1b[:], bcdm[:], Alu.mult)
                nc.vector.tensor_tensor(u[:], u[:], bcbm[:], Alu.add)
                # x0 slot broadcast: [128, KPC, 1->WSLOT, B]
                x0bc = (xslb[:, c * KPC:(c + 1) * KPC, :]
                        .unsqueeze(2).broadcast_to([NPART, KPC, WSLOT, B]))
                u_r = u[:].rearrange("p (k w) b -> p k w b", k=KPC)
                nc.vector.tensor_tensor(u_r, u_r, x0bc, Alu.mult)
                nc.vector.tensor_tensor(v[:], x1b[:], bccm[:], Alu.mult)
                nc.vector.tensor_tensor(v[:], v[:], bca, Alu.add)
                nc.vector.tensor_tensor(v[:], v[:], u[:], Alu.add)

                # segment reduction on PE: per-w 0/1 pairing matrices
                for w in range(W):
                    nc.tensor.matmul(ps[:], pmw_sb[:, w, :], v[:, w, :],
                                     start=(c == 0 and w == 0),
                                     stop=(c == NCHUNK - 1 and w == W - 1))

            out_sb = small_pool.tile([OC, B], f32, tag="out")
            nc.vector.tensor_scalar(out_sb[:], ps[:], 0.25, bias_sb[:],
                                    Alu.mult, Alu.add)
            nc.sync.dma_start(out_d[:], out_sb[:])

    nc.compile()
    return nc


def _wrap(flat):
    """order-j idx list -> dma_gather 16-partition-wrapped [128, n//16]."""
    n = flat.shape[0]
    w = flat.reshape(n // 16, 16).T            # [16, n//16]
    return np.tile(w, (8, 1)).astype(np.int16)  # [128, n//16]


def _host_prep(input, input_mask, lut_weights, bias):
    import ml_dtypes

    input_t = np.ascontiguousarray(input.T).astype(np.float32, copy=False)
    m0 = input_mask[0::2].astype(np.int64)
    m1 = input_mask[1::2].astype(np.int64)
    seg_all = (np.arange(TC) // SEG)

    in_maps = []
    for core in range(NCORES):
        t0 = core * TC
        c0 = m0[t0:t0 + TC]
        c1 = m1[t0:t0 + TC]

        order = np.argsort(c0, kind="stable")
        f_sorted = c0[order]
        starts = np.searchsorted(f_sorted, np.arange(IN), "left")
        pos = np.arange(TC) - starts[f_sorted]
        run_in_f = pos // WSLOT
        w_in = pos % WSLOT
        cnt = np.bincount(c0, minlength=IN)
        nrun = -(-cnt // WSLOT)
        run_base = np.concatenate(([0], np.cumsum(nrun)[:-1]))
        slot = run_base[f_sorted] + run_in_f
        ns = int(nrun.sum())
        assert ns <= NSLOT, f"core {core}: {ns} slots > {NSLOT}"

        p_s = slot % NPART
        k_s = slot // NPART
        wlin = k_s * WSLOT + w_in
        chunk = wlin // W
        wc = wlin % W

        idx1_full = np.zeros((NPART, NCHUNK, W), np.int64)
        lutp = np.zeros((NCHUNK, NPART, W, 4), np.float32)
        pmw = np.zeros((NCHUNK, NPART, W, OC), ml_dtypes.bfloat16)
        idx1_full[p_s, chunk, wc] = c1[order]
        lutp[chunk, p_s, wc] = lut_weights[t0 + order]
        pmw[chunk, p_s, wc, seg_all[order]] = 1.0

        # x1 idx in gather order: per (chunk, sub): j = w_off*128 + p
        cols = []
        for c in range(NCHUNK):
            for s in range(GSUB):
                blk = idx1_full[:, c, s * 8:(s + 1) * 8]      # [128, 8]
                cols.append(_wrap(np.ascontiguousarray(blk.T).reshape(-1)))
        idx1 = np.concatenate(cols, axis=1)

        # x0 slot features; slot s -> gather position j = s
        slot_feat = np.zeros(NSLOT, np.int64)
        slot_feat[:ns] = np.repeat(np.arange(IN), nrun)
        idx0s = np.concatenate(
            [_wrap(slot_feat[0:1024]), _wrap(slot_feat[1024:2048]),
             _wrap(slot_feat[2048:2304])], axis=1)

        in_maps.append({
            "input_t": input_t,
            "idx1": np.ascontiguousarray(idx1),
            "idx0s": np.ascontiguousarray(idx0s),
            "lutp": np.ascontiguousarray(
                lutp.reshape(NCHUNK, NPART, W * 4)),
            "pmw": np.ascontiguousarray(pmw.reshape(NCHUNK, NPART, W * OC)),
            "bias_sh": np.ascontiguousarray(
                bias[core * OC:(core + 1) * OC].reshape(OC, 1)
            ).astype(np.float32, copy=False),
        })
    return in_maps


def get_program():
    if "nc" not in _CACHE:
        _CACHE["nc"] = _build_program()
    return _CACHE["nc"]


def run(input, input_mask, lut_weights, bias, trace=False):
    from concourse.bass_utils import run_bass_kernel_spmd

    nc = get_program()
    in_maps = _host_prep(np.asarray(input), np.asarray(input_mask),
                         np.asarray(lut_weights), np.asarray(bias))
    res = run_bass_kernel_spmd(nc, in_maps, list(range(NCORES)), trace=trace)
    out = np.concatenate([r["out_c"].T for r in res.results], axis=1)
    return out.astype(np.float32, copy=False), res


def kernel(input, input_mask, lut_weights, bias):
    out, _ = run(input, input_mask, lut_weights, bias)
    return out


# revision 23
# speedup vs baseline: 1.0721x; 1.0721x over previous
"""Trainium2 Bass kernel for the LUT-linear (embedding_lookup) problem.

Math: per_table[b,t] = sum_c lut[t,c] * prod_j (1 + s_{c,j} x_j)/2 with
x_0 = input[b, mask[2t]], x_1 = input[b, mask[2t+1]], K=2 (KK=4 corners).
Expanding the corner products (codes s in {-1,+1}):
    per_table = a_t + b_t x0 + c_t x1 + d_t x0 x1
    4a = w0+w1+w2+w3, 4b = -w0+w1-w2+w3, 4c = -w0-w1+w2+w3, 4d = w0-w1-w2+w3
out[b,o] = bias[o] + sum_{t in seg_o} per_table   (segments are 512 contiguous
tables per out-feature).

Device strategy (8 NeuronCores, table-sharded; input replicated). The kernel
is SWDGE descriptor-rate bound (~10-12 ns/descriptor per queue, 4 queues), so
the layout is chosen to minimize descriptor count:
  - Tables are grouped (host-side, index-only planning) into single-feature
    slots of width 16: slot (p, k) holds up to 16 tables sharing one x0
    feature. x0 then needs only one 256B gather descriptor PER SLOT (2304)
    instead of per table (32768); inside the compute it is a stride-0
    broadcast view along the slot width. Only x1 is gathered per table
    (36864 descs incl. slot padding). Total 39168 descs vs 65536 for the
    naive both-gathers layout (201.5us naive -> 160-178us this version,
    rel err 3.3e-3 vs the 2e-2 gate; run-to-run HW variance is +/-15us).
  - Slots mix out-features, so the segment reduction cannot use a single
    partition-pairing matmul. Instead PE does the reduction per w-column
    with 0/1 host-built matrices: psum[o,b] += pmw[:,w,:]^T @ y[:,w,:].
    ldweights+matmul pairs sustain ~53ns on TRN2, 288 pairs ~= 15us.
  - DVE computes y = (d*x1 + b)*x0 + (c*x1 + a) in bf16. The b/c/d
    coefficient broadcasts are materialized as [128, W, B] tiles on the
    otherwise-idle Act engine so the DVE tensor_tensor ops all hit the
    2x 16-bit mode (a stride-0 last-dim broadcast operand forces 1x).
    Act also does the f32->bf16 casts (contiguous only - transposed or
    strided-column writes on Act/DVE are 3-5x slower). 18 half-size
    chunks (W=16) shrink the pipeline fill/drain tails.
  - Host does only index planning / layout transforms (permute, pad, cast,
    one-hot from indices); all value arithmetic stays on device.
"""

import numpy as np

NCORES = 8
B = 64
IN = 512
OUT = 512
T = IN * OUT
TC = T // NCORES          # tables per core = 32768
SEG = 512                 # tables per out-feature
OC = OUT // NCORES        # out-features per core = 64
NPART = 128

# slot layout
WSLOT = 16                # tables per single-feature slot
NPASS = 18                # slot columns per partition (worst core needs 2299 slots)
NSLOT = NPASS * NPART     # 2304 slots
WT2 = NPASS * WSLOT       # 288 w-positions per partition
NCHUNK = 9
W = WT2 // NCHUNK         # 32 w per chunk (= 2 passes)
KPC = NPASS // NCHUNK           # slot columns per chunk = 2
GIDX = 1024               # indices per dma_gather (ucode limit)
GSUB = NPART * W // GIDX  # x1 sub-gathers per chunk = 4
NQUEUES = 4

_CACHE = {}


def _build_program():
    import concourse.bacc as bacc
    import concourse.mybir as mybir
    from concourse import library_config
    from concourse.tile import TileContext

    f32 = mybir.dt.float32
    bf16 = mybir.dt.bfloat16
    i16 = mybir.dt.int16
    Alu = mybir.AluOpType
    SP = False

    nc = bacc.Bacc("TRN2", target_bir_lowering=False, debug=False,
                   num_devices=NCORES, num_swdge_queues=NQUEUES,
                   dynamic_dma_scratch_size=32768)

    input_t = nc.dram_tensor("input_t", [IN, B], f32, kind="ExternalInput")
    # x1 idx: per (chunk, sub) 64 wrapped cols
    idx1_d = nc.dram_tensor("idx1", [NPART, NCHUNK * GSUB * (GIDX // 16)],
                            i16, kind="ExternalInput")
    # x0 slot idx: 1024+1024+256 idxs -> 64+64+16 wrapped cols
    idx0s_d = nc.dram_tensor("idx0s", [NPART, 144], i16, kind="ExternalInput")
    lutp_d = nc.dram_tensor("lutp", [NCHUNK, NPART, W * 4], f32,
                            kind="ExternalInput")
    pmw_d = nc.dram_tensor("pmw", [NCHUNK, NPART, W * OC], bf16,
                           kind="ExternalInput")
    bias_d = nc.dram_tensor("bias_sh", [OC, 1], f32, kind="ExternalInput")
    out_d = nc.dram_tensor("out_c", [OC, B], f32, kind="ExternalOutput")

    with TileContext(nc) as tc:
        nc.gpsimd.load_library(library_config.mlp)
        with (
            tc.tile_pool(name="idx", bufs=1) as idx_pool,
            tc.tile_pool(name="small", bufs=1) as small_pool,
            tc.tile_pool(name="lut", bufs=2) as lut_pool,
            tc.tile_pool(name="pmw", bufs=2) as pmw_pool,
            tc.tile_pool(name="coef", bufs=2) as coef_pool,
            tc.tile_pool(name="xs", bufs=1) as xs_pool,
            tc.tile_pool(name="x1", bufs=4) as x1_pool,
            tc.tile_pool(name="m", bufs=3) as m_pool,
            tc.tile_pool(name="cm", bufs=2) as cm_pool,
            tc.tile_pool(name="psum", bufs=1, space="PSUM") as psum_pool,
        ):
            idx1_sb = idx_pool.tile([NPART, NCHUNK * GSUB * 64], i16, tag="idx1")
            idx0s_sb = idx_pool.tile([NPART, 144], i16, tag="idx0s")
            nc.sync.dma_start(idx1_sb[:], idx1_d[:])
            nc.sync.dma_start(idx0s_sb[:], idx0s_d[:])

            # x0 slot values: one descriptor per slot
            xsl = xs_pool.tile([NPART, NPASS, B], f32, tag="xsl")
            nc.gpsimd.dma_gather(xsl[:, 0:8, :], input_t[:],
                                 idx0s_sb[:, 0:64], GIDX, GIDX, B,
                                 queue_num=0, single_packet=SP)
            nc.gpsimd.dma_gather(xsl[:, 8:16, :], input_t[:],
                                 idx0s_sb[:, 64:128], GIDX, GIDX, B,
                                 queue_num=1, single_packet=SP)
            nc.gpsimd.dma_gather(xsl[:, 16:18, :], input_t[:],
                                 idx0s_sb[:, 128:144], 256, 256, B,
                                 queue_num=2, single_packet=SP)
            xslb = xs_pool.tile([NPART, NPASS, B], bf16, tag="xslb")
            nc.scalar.copy(xslb[:], xsl[:])

            bias_sb = small_pool.tile([OC, 1], f32, tag="bias")
            nc.sync.dma_start(bias_sb[:], bias_d[:])

            ps = psum_pool.tile([OC, B], f32, tag="ps")

            for c in range(NCHUNK):
                w4 = lut_pool.tile([NPART, W, 4], f32, tag="w4")
                nc.sync.dma_start(w4[:], lutp_d[c].rearrange("p (w k) -> p w k", k=4))
                pmw_sb = pmw_pool.tile([NPART, W, OC], bf16, tag="pmw")
                nc.sync.dma_start(pmw_sb[:], pmw_d[c].rearrange("p (w o) -> p w o", o=OC))

                # coefficient transform -> bf16 (values are 4x true a,b,c,d;
                # folded back by the 0.25 output scale)
                ca = coef_pool.tile([NPART, W], bf16, tag="ca")
                cb = coef_pool.tile([NPART, W], bf16, tag="cb")
                cc = coef_pool.tile([NPART, W], bf16, tag="cc")
                cd = coef_pool.tile([NPART, W], bf16, tag="cd")
                t1 = coef_pool.tile([NPART, W], f32, tag="t1")
                t2 = coef_pool.tile([NPART, W], f32, tag="t2")
                nc.vector.tensor_tensor(t1[:], w4[:, :, 0], w4[:, :, 3], Alu.add)
                nc.vector.tensor_tensor(t2[:], w4[:, :, 1], w4[:, :, 2], Alu.add)
                nc.vector.tensor_tensor(ca[:], t1[:], t2[:], Alu.add)
                nc.vector.tensor_tensor(cd[:], t1[:], t2[:], Alu.subtract)
                nc.vector.tensor_tensor(t1[:], w4[:, :, 3], w4[:, :, 0], Alu.subtract)
                nc.vector.tensor_tensor(t2[:], w4[:, :, 1], w4[:, :, 2], Alu.subtract)
                nc.vector.tensor_tensor(cb[:], t1[:], t2[:], Alu.add)
                nc.vector.tensor_tensor(cc[:], t1[:], t2[:], Alu.subtract)

                # x1 gathers (4 x 1024 descs)
                x1 = x1_pool.tile([NPART, W, B], f32, tag="x1")
                for j in range(GSUB):
                    q = (c * GSUB + j) % NQUEUES
                    nc.gpsimd.dma_gather(
                        x1[:, j * 8:(j + 1) * 8, :], input_t[:],
                        idx1_sb[:, (c * GSUB + j) * 64:(c * GSUB + j + 1) * 64], GIDX, GIDX, B,
                        queue_num=q, single_packet=SP)

                x1b = m_pool.tile([NPART, W, B], bf16, tag="x1b")
                nc.scalar.copy(x1b[:], x1[:])

                # Act materializes the two hot coefficient broadcasts so
                # their DVE ops run in the 2x 16-bit mode.
                bcdm = cm_pool.tile([NPART, W, B], bf16, tag="bcdm")
                bccm = cm_pool.tile([NPART, W, B], bf16, tag="bccm")
                bcbm = cm_pool.tile([NPART, W, B], bf16, tag="bcbm")
                nc.scalar.copy(bcdm[:], cd[:].unsqueeze(2).broadcast_to([NPART, W, B]))
                nc.scalar.copy(bccm[:], cc[:].unsqueeze(2).broadcast_to([NPART, W, B]))
                nc.scalar.copy(bcbm[:], cb[:].unsqueeze(2).broadcast_to([NPART, W, B]))

                # y = (d*x1 + b) * x0 + c*x1
                u = m_pool.tile([NPART, W, B], bf16, tag="u")
                v = m_pool.tile([NPART, W, B], bf16, tag="v")
                bca = ca[:].unsqueeze(2).broadcast_to([NPART, W, B])
                nc.vector.tensor_tensor(u[:], x1b[:], bcdm[:], Alu.mult)
                nc.vector.tensor_tensor(u[:], u[:], bcbm[:], Alu.add)
                # x0 slot broadcast: [128, KPC, 1->WSLOT, B]
                x0bc = (xslb[:, c * KPC:(c + 1) * KPC, :]
                        .unsqueeze(2).broadcast_to([NPART, KPC, WSLOT, B]))
                u_r = u[:].rearrange("p (k w) b -> p k w b", k=KPC)
                nc.vector.tensor_tensor(u_r, u_r, x0bc, Alu.mult)
                nc.vector.tensor_tensor(v[:], x1b[:], bccm[:], Alu.mult)
                nc.vector.tensor_tensor(v[:], v[:], bca, Alu.add)
                nc.vector.tensor_tensor(v[:], v[:], u[:], Alu.add)

                # segment reduction on PE: per-w 0/1 pairing matrices
                for w in range(W):
                    nc.tensor.matmul(ps[:], pmw_sb[:, w, :], v[:, w, :],
                                     start=(c == 0 and w == 0),
                                     stop=(c == NCHUNK - 1 and w == W - 1))

            out_sb = small_pool.tile([OC, B], f32, tag="out")
            nc.vector.tensor_scalar(out_sb[:], ps[:], 0.25, bias_sb[:],
                                    Alu.mult, Alu.add)
            nc.sync.dma_start(out_d[:], out_sb[:])

    nc.compile()
    return nc


def _wrap(flat):
    """order-j idx list -> dma_gather 16-partition-wrapped [128, n//16]."""
    n = flat.shape[0]
    w = flat.reshape(n // 16, 16).T            # [16, n//16]
    return np.tile(w, (8, 1)).astype(np.int16)  # [128, n//16]


def _host_prep(input, input_mask, lut_weights, bias):
    import ml_dtypes

    input_t = np.ascontiguousarray(input.T).astype(np.float32, copy=False)
    m0 = input_mask[0::2].astype(np.int64)
    m1 = input_mask[1::2].astype(np.int64)
    seg_all = (np.arange(TC) // SEG)

    in_maps = []
    for core in range(NCORES):
        t0 = core * TC
        c0 = m0[t0:t0 + TC]
        c1 = m1[t0:t0 + TC]

        order = np.argsort(c0, kind="stable")
        f_sorted = c0[order]
        starts = np.searchsorted(f_sorted, np.arange(IN), "left")
        pos = np.arange(TC) - starts[f_sorted]
        run_in_f = pos // WSLOT
        w_in = pos % WSLOT
        cnt = np.bincount(c0, minlength=IN)
        nrun = -(-cnt // WSLOT)
        run_base = np.concatenate(([0], np.cumsum(nrun)[:-1]))
        slot = run_base[f_sorted] + run_in_f
        ns = int(nrun.sum())
        assert ns <= NSLOT, f"core {core}: {ns} slots > {NSLOT}"

        p_s = slot % NPART
        k_s = slot // NPART
        wlin = k_s * WSLOT + w_in
        chunk = wlin // W
        wc = wlin % W

        idx1_full = np.zeros((NPART, NCHUNK, W), np.int64)
        lutp = np.zeros((NCHUNK, NPART, W, 4), np.float32)
        pmw = np.zeros((NCHUNK, NPART, W, OC), ml_dtypes.bfloat16)
        idx1_full[p_s, chunk, wc] = c1[order]
        lutp[chunk, p_s, wc] = lut_weights[t0 + order]
        pmw[chunk, p_s, wc, seg_all[order]] = 1.0

        # x1 idx in gather order: per (chunk, sub): j = w_off*128 + p
        cols = []
        for c in range(NCHUNK):
            for s in range(GSUB):
                blk = idx1_full[:, c, s * 8:(s + 1) * 8]      # [128, 8]
                cols.append(_wrap(np.ascontiguousarray(blk.T).reshape(-1)))
        idx1 = np.concatenate(cols, axis=1)

        # x0 slot features; slot s -> gather position j = s
        slot_feat = np.zeros(NSLOT, np.int64)
        slot_feat[:ns] = np.repeat(np.arange(IN), nrun)
        idx0s = np.concatenate(
            [_wrap(slot_feat[0:1024]), _wrap(slot_feat[1024:2048]),
             _wrap(slot_feat[2048:2304])], axis=1)

        in_maps.append({
            "input_t": input_t,
            "idx1": np.ascontiguousarray(idx1),
            "idx0s": np.ascontiguousarray(idx0s),
            "lutp": np.ascontiguousarray(
                lutp.reshape(NCHUNK, NPART, W * 4)),
            "pmw": np.ascontiguousarray(pmw.reshape(NCHUNK, NPART, W * OC)),
            "bias_sh": np.ascontiguousarray(
                bias[core * OC:(core + 1) * OC].reshape(OC, 1)
            ).astype(np.float32, copy=False),
        })
    return in_maps


def get_program():
    if "nc" not in _CACHE:
        _CACHE["nc"] = _build_program()
    return _CACHE["nc"]


def run(input, input_mask, lut_weights, bias, trace=False):
    from concourse.bass_utils import run_bass_kernel_spmd

    nc = get_program()
    in_maps = _host_prep(np.asarray(input), np.asarray(input_mask),
                         np.asarray(lut_weights), np.asarray(bias))
    res = run_bass_kernel_spmd(nc, in_maps, list(range(NCORES)), trace=trace)
    out = np.concatenate([r["out_c"].T for r in res.results], axis=1)
    return out.astype(np.float32, copy=False), res


def kernel(input, input_mask, lut_weights, bias):
    out, _ = run(input, input_mask, lut_weights, bias)
    return out
